# revision 2
# baseline (speedup 1.0000x reference)
"""GATv2 layer (N=50000, D=128, H=4, E=600000) on 8 trn2 NeuronCores — v2.

Key differences from the baseline:
- Per-window bulk dma_gather (SWDGE) instead of per-chunk indirect DMAs:
  3 gather calls per 4-window group instead of ~2 per 128-edge chunk.
- xl table split into two <=25088-row halves (dma_gather int16 indices);
  each window's edges sorted low-src-first so each half gathers a
  contiguous chunk range (pads gather row 0, excluded via sel==0).
- xl and xr gathered separately; aggregation uses alpha*xl directly
  (baseline's sum(alpha*y) - xr cancellation is gone).
- Host supplies transposed h (bf16); phase-1 matmuls are bf16 (1 cyc/row)
  with f32 PSUM accumulate; tables stored bf16.
- Edge pipeline in bf16 (2x DVE), scores reduced in f32, softmax/normalize
  in f32; scatter matmuls bf16 with f32 PSUM accumulation.
- Per-window chunk counts K_w baked from the actual edge data (max over
  cores so the program is shared), instead of a global max K.
"""

import math
import numpy as np

import concourse.bass as bass
import concourse.bacc as bacc
import concourse.mybir as mybir
import concourse.tile as tile
from concourse.masks import make_identity
from concourse.bass_utils import run_bass_kernel_spmd

P = 128
F32 = mybir.dt.float32
BF16 = mybir.dt.bfloat16
I16 = mybir.dt.int16
I32 = mybir.dt.int32

NEG_SLOPE = 0.2
BN_EPS = 1e-5
SPLIT = 25088           # node id split so gather indices fit int16
GROUP = 4               # windows per gather group
PAD_REL = 300.0         # dst-rel sentinel for padded edge slots


def _wrap16(arr):
    """[n] ints -> [128, n//16] int16 wrapped (i -> [i%16, i//16]),
    replicated across the 8 gpsimd cores."""
    n = len(arr)
    assert n % 16 == 0
    w = np.asarray(arr, np.int16).reshape(n // 16, 16).T.copy()
    return np.tile(w, (8, 1))


def _to_bf16_bits(f32arr):
    return (np.asarray(f32arr, np.float32).view(np.uint32) >> 16).astype(
        np.uint16).view(np.int16)


class Meta:
    pass


def host_prepare(h, edge_index, W_l, W_r, bias_l, bias_r, att,
                 bias_out, gamma, beta, n_cores=8):
    N, D = h.shape
    H, C = np.asarray(att).shape
    assert D == P
    h = np.asarray(h, np.float32)
    NT = math.ceil(N / P)
    NTA = SPLIT // P
    NTB = NT - NTA
    NPC = N // n_cores
    W = math.ceil(NPC / P)
    LASTR = NPC - P * (W - 1)

    loops = np.arange(N, dtype=np.int64)
    src = np.concatenate([np.asarray(edge_index[0]), loops]).astype(np.int64)
    dst = np.concatenate([np.asarray(edge_index[1]), loops]).astype(np.int64)
    order = np.argsort(dst, kind="stable")
    src_s = src[order].astype(np.int32)
    dst_s = dst[order].astype(np.int32)
    bounds = np.searchsorted(dst_s, np.arange(0, N + 1, NPC))

    # per (core, window) split into low/high src
    percore = []
    CL = np.zeros(W, np.int64)
    CH = np.zeros(W, np.int64)
    for k in range(n_cores):
        lo, hi = bounds[k], bounds[k + 1]
        s_k = src_s[lo:hi]
        d_k = dst_s[lo:hi] - k * NPC
        win = d_k // P
        wb = np.searchsorted(win, np.arange(W + 1))
        rows = []
        for w in range(W):
            a, b = wb[w], wb[w + 1]
            sw = s_k[a:b]
            rw = (d_k[a:b] - w * P).astype(np.float32)
            dl = d_k[a:b]
            low = sw < SPLIT
            rows.append((sw[low], rw[low], dl[low],
                         sw[~low] - SPLIT, rw[~low], dl[~low]))
            CL[w] = max(CL[w], (len(sw[low]) + P - 1) // P)
            CH[w] = max(CH[w], (len(sw[~low]) + P - 1) // P)
        percore.append(rows)

    # groups of windows
    groups = []
    c0 = 0
    for g0 in range(0, W, GROUP):
        ws = list(range(g0, min(g0 + GROUP, W)))
        SA = int(sum(CL[w] for w in ws))
        SB = int(sum(CH[w] for w in ws))
        SK = SA + SB
        grp = Meta()
        grp.ws = ws
        grp.SA, grp.SB, grp.SK = SA, SB, SK
        grp.c0 = c0
        grp.cols = SA * 8 + SB * 8 + SK
        # per-window offsets
        grp.aoff = {}
        grp.boff = {}
        ao = bo = 0
        for w in ws:
            grp.aoff[w] = ao
            grp.boff[w] = bo
            ao += int(CL[w])
            bo += int(CH[w])
        c0 += grp.cols
        groups.append(grp)
    TOT = c0

    meta = Meta()
    meta.N, meta.H, meta.C = N, H, C
    meta.NT, meta.NTA, meta.NTB = NT, NTA, NTB
    meta.NPC, meta.W, meta.LASTR = NPC, W, LASTR
    meta.n_cores = n_cores
    meta.CL, meta.CH = CL, CH
    meta.groups = groups
    meta.TOT = TOT
    meta.BMAX = int(max(max(CL.max(), CH.max()), 1))
    meta.SAMAX = int(max(g.SA for g in groups))
    meta.SBMAX = int(max(g.SB for g in groups))
    meta.zero_bias = (not np.any(np.asarray(bias_l))
                      and not np.any(np.asarray(bias_r)))
    meta.zero_bias_out = not np.any(np.asarray(bias_out))
    assert meta.zero_bias and meta.zero_bias_out, \
        "non-zero bias path not implemented"

    # ---- global tensors ----
    hpadT = np.zeros((P, NT * P), np.float32)
    hpadT[:, :N] = h.T
    hT16 = _to_bf16_bits(hpadT)

    wlr = np.zeros((P, 256), np.float32)
    wlr[:, 0:P] = np.asarray(W_l, np.float32)
    wlr[:, P:256] = np.asarray(W_r, np.float32)
    wlr16 = _to_bf16_bits(wlr)

    cvec = np.zeros((P, 3), np.float32)
    cvec[:, 0] = np.asarray(att, np.float32).reshape(-1)
    cvec[:, 1] = np.asarray(gamma, np.float32)
    cvec[:, 2] = np.asarray(beta, np.float32)

    in_maps = []
    for k in range(n_cores):
        rows = percore[k]
        idxpk = np.zeros((P, TOT), np.int16)
        for grp in groups:
            acols = []
            bcols = []
            rcols = []
            relv = []
            for w in grp.ws:
                sl, rl, dl, sh, rh, dh = rows[w]
                nA = int(CL[w]) * P
                nB = int(CH[w]) * P
                sA = np.zeros(nA, np.int32)
                sA[:len(sl)] = sl
                rA = np.full(nA, PAD_REL, np.float32)
                rA[:len(rl)] = rl
                dA = np.zeros(nA, np.int32)
                dA[:len(dl)] = dl
                acols.append(sA)
                rcolA = dA
                sB = np.zeros(nB, np.int32)
                sB[:len(sh)] = sh
                rB = np.full(nB, PAD_REL, np.float32)
                rB[:len(rh)] = rh
                dB = np.zeros(nB, np.int32)
                dB[:len(dh)] = dh
                bcols.append(sB)
                relv.append((rA, rB))
                rcols.append((rcolA, dB))
            astream = np.concatenate(acols) if acols else np.zeros(0, np.int32)
            bstream = np.concatenate(bcols) if bcols else np.zeros(0, np.int32)
            relstream = np.concatenate([rv[0] for rv in relv]
                                       + [rv[1] for rv in relv])
            co = grp.c0
            if grp.SA:
                idxpk[:, co:co + grp.SA * 8] = _wrap16(astream)
            co += grp.SA * 8
            if grp.SB:
                idxpk[:, co:co + grp.SB * 8] = _wrap16(bstream)
            co += grp.SB * 8
            # rel as bf16 bits, laid out [P, SK] (slot s*128+p -> [p, s])
            rel16 = _to_bf16_bits(relstream.reshape(grp.SK, P).T.copy())
            idxpk[:, co:co + grp.SK] = rel16
            co += grp.SK
            assert co == grp.c0 + grp.cols

        hloc = np.zeros((W * P, P), np.float32)
        hloc[:NPC] = h[k * NPC:(k + 1) * NPC]
        hlocT16 = _to_bf16_bits(
            np.vstack([hloc.T]))  # [P, W*P]

        in_maps.append({
            "hT16": hT16, "hlocT16": hlocT16, "hloc": hloc,
            "wlr16": wlr16, "cvec": cvec, "idxpk": idxpk,
        })
    return meta, in_maps


def build_kernel(nc: bass.Bass, m: Meta, mode: str = "full"):
    H, C = m.H, m.C
    W, NT, NTA, NTB = m.W, m.NT, m.NTA, m.NTB
    BMAX = m.BMAX
    DEN = P + H

    # ---- I/O ----
    hT16 = nc.declare_dram_parameter("hT16", [P, NT * P], I16, isOutput=False)
    hlocT16 = nc.declare_dram_parameter("hlocT16", [P, W * P], I16,
                                        isOutput=False)
    hloc = nc.declare_dram_parameter("hloc", [W * P, P], F32, isOutput=False)
    wlr16 = nc.declare_dram_parameter("wlr16", [P, 256], I16, isOutput=False)
    cvec = nc.declare_dram_parameter("cvec", [P, 3], F32, isOutput=False)
    idxpk = nc.declare_dram_parameter("idxpk", [P, m.TOT], I16, isOutput=False)
    out = nc.declare_dram_parameter("out", [m.NPC, P], F32, isOutput=True)

    # ---- internal DRAM ----
    xl_tabA = nc.dram_tensor("xl_tabA", [NTA * P, P], BF16)
    xl_tabB = nc.dram_tensor("xl_tabB", [NTB * P, P], BF16)
    xr_tab = nc.dram_tensor("xr_tab", [W * P, P], BF16)
    st_in = nc.dram_tensor("st_in", [P, 2], F32)
    st_out = nc.dram_tensor("st_out", [P, 2], F32, addr_space="Shared")

    with tile.TileContext(nc) as tc:
        import contextlib
        with contextlib.ExitStack() as ctx:
            cst = ctx.enter_context(tc.tile_pool(name="cst", bufs=1))
            sbA = ctx.enter_context(tc.tile_pool(name="sbA", bufs=3))
            sbI = ctx.enter_context(tc.tile_pool(name="sbI", bufs=3))
            sbG = ctx.enter_context(tc.tile_pool(name="sbG", bufs=3))
            sbB = ctx.enter_context(tc.tile_pool(name="sbB", bufs=3))
            sbS = ctx.enter_context(tc.tile_pool(name="sbS", bufs=3))
            psT = ctx.enter_context(tc.tile_pool(name="psT", bufs=1,
                                                 space="PSUM"))
            psU = ctx.enter_context(tc.tile_pool(name="psU", bufs=2,
                                                 space="PSUM"))
            psX = ctx.enter_context(tc.tile_pool(name="psX", bufs=2,
                                                 space="PSUM"))
            psw = ctx.enter_context(tc.tile_pool(name="psw", bufs=2,
                                                 space="PSUM"))
            ps1 = ctx.enter_context(tc.tile_pool(name="ps1", bufs=1,
                                                 space="PSUM"))

            # ================= constants =================
            wlr_sb = cst.tile([P, 256], BF16, tag="wlr")
            nc.sync.dma_start(out=wlr_sb[:], in_=wlr16[:].bitcast(BF16))
            cv = cst.tile([P, 3], F32, tag="cv")
            nc.sync.dma_start(out=cv[:], in_=cvec[:])
            att_col = cv[:, 0:1]
            gam_col = cv[:, 1:2]
            bet_col = cv[:, 2:3]

            ident = cst.tile([P, P], F32, tag="ident")
            make_identity(nc, ident[:])
            ident16 = cst.tile([P, P], BF16, tag="ident16")
            nc.vector.tensor_copy(ident16[:], ident[:])
            ones_col = cst.tile([P, 1], F32, tag="ones_c")
            nc.gpsimd.memset(ones_col[:], 1.0)
            eps_col = cst.tile([P, 1], F32, tag="epsc")
            nc.gpsimd.memset(eps_col[:], BN_EPS)

            iota_i = cst.tile([P, BMAX * P], I32, tag="iota_i")
            nc.gpsimd.iota(iota_i[:], pattern=[[0, BMAX], [1, P]],
                           channel_multiplier=0)
            iota3 = cst.tile([P, BMAX, P], BF16, tag="iota3")
            nc.vector.tensor_copy(
                iota3[:].rearrange("p a c -> p (a c)"), iota_i[:])

            att_ps = psT.tile([P, P], F32, tag="pt")
            nc.tensor.transpose(att_ps[:], att_col.to_broadcast([P, P]),
                                ident[:])
            att_rep = cst.tile([P, P], BF16, tag="attrep")
            nc.scalar.copy(att_rep[:], att_ps[:])
            att3 = cst.tile([P, BMAX, P], BF16, tag="att3")
            nc.scalar.copy(att3[:],
                           att_rep[:, None, :].to_broadcast([P, BMAX, P]))

            # ================= phase 1: xl tables (all nodes) ============
            # NTA = 196 is a multiple of 8? 196 % 8 == 4, so slabs of 8 must
            # not straddle the A/B boundary: NTA % 4 == 0 and we emit slabs
            # aligned to 4-tile boundaries inside one table at a time.
            assert NTA % 4 == 0

            def p1_build(src_ap, dst_tab, ntt, wslice):
                NSLAB = math.ceil(ntt / 8)
                for sl in range(NSLAB):
                    t0 = sl * 8
                    ntile = min(8, ntt - t0)
                    ncols = ntile * P
                    slab = sbA.tile([P, 8 * P], BF16, tag="slab")
                    nc.sync.dma_start(
                        out=slab[:, :ncols],
                        in_=src_ap[:, t0 * P:t0 * P + ncols].bitcast(BF16))
                    xs = sbA.tile([P, 8, P], BF16, tag="xs")
                    for j0 in range(0, ntile, 2):
                        np_ = min(2, ntile - j0)
                        px = psX.tile([P, 4, P], F32, tag="xre")
                        pxf = px[:].rearrange("p q c -> p (q c)")
                        for j in range(np_):
                            nc.tensor.matmul(
                                pxf[:, j * 256:j * 256 + wslice.free_size()],
                                lhsT=slab[:, (j0 + j) * P:(j0 + j + 1) * P],
                                rhs=wslice, start=True, stop=True)
                        nc.scalar.copy(
                            xs[:, j0:j0 + np_, :],
                            px[:].rearrange("p (u v) c -> p u v c",
                                            u=2)[:, :np_, 0, :])
                    nc.sync.dma_start(
                        out=dst_tab[t0 * P:(t0 + ntile) * P, :]
                            .rearrange("(j p) c -> p j c", j=ntile),
                        in_=xs[:, :ntile, :])

            p1_build(hT16[:, 0:NTA * P], xl_tabA, NTA, wlr_sb[:])
            p1_build(hT16[:, NTA * P:NT * P], xl_tabB, NTB, wlr_sb[:])
            p1_build(hlocT16, xr_tab, W, wlr_sb[:, P:256])

            tc.strict_bb_all_engine_barrier()

            if mode == "xr":
                for w in range(W):
                    rows = P if w < W - 1 else m.LASTR
                    xv = sbS.tile([P, P], BF16, tag="xv")
                    nc.sync.dma_start(out=xv[:],
                                      in_=xr_tab[w * P:(w + 1) * P, :])
                    xf = sbS.tile([P, P], F32, tag="xf")
                    nc.vector.tensor_copy(xf[:], xv[:])
                    nc.sync.dma_start(out=out[w * P:w * P + rows, :],
                                      in_=xf[:rows, :])
                return nc

            if mode == "p1":
                for w in range(W):
                    rows = P if w < W - 1 else m.LASTR
                    hres = sbS.tile([P, P], F32, tag="hres")
                    nc.sync.dma_start(out=hres[:],
                                      in_=hloc[w * P:(w + 1) * P, :])
                    nc.sync.dma_start(out=out[w * P:w * P + rows, :],
                                      in_=hres[:rows, :])
                return nc

            # ================= phase 2: edges =================
            stats_ps = ps1.tile([P, 2], F32, tag="stats")
            outpre = []
            for w in range(W):
                op_w = cst.tile([P, P], F32, tag=f"op{w}")
                outpre.append(op_w)

            for grp in m.groups:
                it = sbI.tile([P, max(g.cols for g in m.groups)], I16,
                              tag="idx")
                nc.sync.dma_start(out=it[:, :grp.cols],
                                  in_=idxpk[:, grp.c0:grp.c0 + grp.cols])
                G1A = sbG.tile([P, max(m.SAMAX, 1), P], BF16, tag="g1a")
                G1B = sbG.tile([P, max(m.SBMAX, 1), P], BF16, tag="g1b")
                a0 = 0
                b0 = grp.SA * 8
                q0 = b0 + grp.SB * 8
                if grp.SA:
                    nc.gpsimd.dma_gather(
                        out_ap=G1A[:, :grp.SA, :], in_ap=xl_tabA[:],
                        idxs_ap=it[:, a0:a0 + grp.SA * 8],
                        num_idxs=grp.SA * P, num_idxs_reg=grp.SA * P,
                        elem_size=P, single_packet=False)
                if grp.SB:
                    nc.gpsimd.dma_gather(
                        out_ap=G1B[:, :grp.SB, :], in_ap=xl_tabB[:],
                        idxs_ap=it[:, b0:b0 + grp.SB * 8],
                        num_idxs=grp.SB * P, num_idxs_reg=grp.SB * P,
                        elem_size=P, single_packet=False)
                rrg = it[:, q0:q0 + grp.SK].bitcast(BF16)

                if mode == "gather":
                    for w in grp.ws:
                        op_w = outpre[w]
                        nc.vector.tensor_copy(
                            op_w[:], G1A[:, grp.aoff[w], :])
                    continue

                for w in grp.ws:
                    blocks = []
                    if m.CL[w]:
                        blocks.append((G1A, grp.aoff[w], int(m.CL[w]),
                                       grp.aoff[w]))
                    if m.CH[w]:
                        blocks.append((G1B, grp.boff[w], int(m.CH[w]),
                                       grp.SA + grp.boff[w]))
                    xr_win = sbS.tile([P, P], BF16, tag="xrw")
                    nc.sync.dma_start(out=xr_win[:],
                                      in_=xr_tab[w * P:(w + 1) * P, :])
                    wps = psw.tile([P, DEN], F32, tag="wps")
                    nch = sum(b[2] for b in blocks)
                    ji = 0
                    for (Gt, goff, ncb, roff) in blocks:
                        Gx = Gt[:, goff:goff + ncb, :]
                        rrx = rrg[:, roff:roff + ncb]
                        # sel[e, d] = (dst_e == d); also drives xr expansion
                        sel = sbB.tile([P, BMAX, P], BF16, tag="sel")
                        nc.vector.tensor_tensor(
                            out=sel[:, :ncb, :],
                            in0=rrx[:, :, None].to_broadcast([P, ncb, P]),
                            in1=iota3[:, :ncb, :],
                            op=mybir.AluOpType.is_equal)
                        selT = sbB.tile([P, BMAX, P], BF16, tag="selT")
                        z = sbB.tile([P, BMAX, P], BF16, tag="z")
                        done = 0
                        while done < ncb:
                            nq = min(4, ncb - done)
                            stq = psU.tile([P, 4, P], BF16, tag="pt16")
                            for j in range(nq):
                                nc.tensor.transpose(stq[:, j, :],
                                                    sel[:, done + j, :],
                                                    ident16[:])
                            nc.scalar.copy(selT[:, done:done + nq, :],
                                           stq[:, :nq, :])
                            # y = xr[dst] + xl[src], accumulated in PSUM:
                            # selT@xr_win then += I@Gx
                            xre = psX.tile([P, 4, P], F32, tag="xre")
                            for j in range(nq):
                                nc.tensor.matmul(
                                    xre[:, j, :],
                                    lhsT=selT[:, done + j, :],
                                    rhs=xr_win[:], start=True, stop=False)
                                nc.tensor.matmul(
                                    xre[:, j, :],
                                    lhsT=ident16[:],
                                    rhs=Gx[:, done + j, :],
                                    start=False, stop=True)
                            ab = sbB.tile([P, 4, P], BF16, tag="ab")
                            nc.scalar.activation(
                                ab[:, :nq, :], xre[:, :nq, :],
                                mybir.ActivationFunctionType.Abs,
                                scale=(1.0 - NEG_SLOPE) / 2.0)
                            nc.vector.scalar_tensor_tensor(
                                out=z[:, done:done + nq, :],
                                in0=xre[:, :nq, :],
                                scalar=(1.0 + NEG_SLOPE) / 2.0,
                                in1=ab[:, :nq, :],
                                op0=mybir.AluOpType.mult,
                                op1=mybir.AluOpType.add)
                            done += nq
                        zz = sbB.tile([P, BMAX, P], BF16, tag="zz")
                        nc.vector.tensor_mul(zz[:, :ncb, :], z[:, :ncb, :],
                                             att3[:, :ncb, :])
                        s16 = sbS.tile([P, BMAX * H], F32, tag="s16")
                        nc.vector.tensor_reduce(
                            out=s16[:, :ncb * H].rearrange(
                                "p (a h) -> p a h", h=H)[:, :, :, None],
                            in_=zz[:, :ncb, :].rearrange(
                                "p a (h c) -> p a h c", c=C),
                            op=mybir.AluOpType.add,
                            axis=mybir.AxisListType.X)
                        rhs = sbB.tile([P, BMAX, DEN], BF16, tag="rhs")
                        nc.scalar.activation(
                            rhs[:, :ncb, P:DEN],
                            s16[:, :ncb * H].rearrange("p (a h) -> p a h",
                                                       h=H),
                            mybir.ActivationFunctionType.Exp)
                        nc.vector.tensor_mul(
                            rhs[:, :ncb, 0:P].rearrange(
                                "p a (h c) -> p a h c", c=C),
                            Gx.rearrange("p a (h c) -> p a h c", c=C),
                            rhs[:, :ncb, P:DEN][:, :, :, None]
                                .to_broadcast([P, ncb, H, C]))
                        for j in range(ncb):
                            nc.tensor.matmul(
                                wps[:], lhsT=sel[:, j, :], rhs=rhs[:, j, :],
                                start=(ji == 0), stop=(ji == nch - 1))
                            ji += 1

                    # normalize window
                    dmx = sbS.tile([P, H], F32, tag="dmx")
                    nc.vector.tensor_scalar_max(dmx[:], wps[:, P:DEN], 1e-30)
                    rec = sbS.tile([P, H], F32, tag="rec")
                    nc.vector.reciprocal(rec[:], dmx[:])
                    op_w = outpre[w]
                    nc.vector.tensor_mul(
                        op_w[:].rearrange("p (h c) -> p h c", c=C),
                        wps[:, 0:P].rearrange("p (h c) -> p h c", c=C),
                        rec[:, :, None].to_broadcast([P, H, C]))
                    sq = sbS.tile([P, P], F32, tag="sq")
                    nc.scalar.square(sq[:], op_w[:])
                    nc.tensor.matmul(stats_ps[:, 0:1], lhsT=op_w[:],
                                     rhs=ones_col[:],
                                     start=(w == 0), stop=(w == W - 1))
                    nc.tensor.matmul(stats_ps[:, 1:2], lhsT=sq[:],
                                     rhs=ones_col[:],
                                     start=(w == 0), stop=(w == W - 1))

            if mode == "gather":
                for w in range(W):
                    rows = P if w < W - 1 else m.LASTR
                    nc.sync.dma_start(out=out[w * P:w * P + rows, :],
                                      in_=outpre[w][:rows, :])
                return nc

            # ================= phase 3: BN stats AllReduce ===============
            st_sb = sbS.tile([P, 2], F32, tag="stsb")
            nc.scalar.copy(st_sb[:], stats_ps[:])
            if mode == "nocc":
                st_all = st_sb
                nscale = m.N / m.n_cores
            else:
                nc.sync.dma_start(out=st_in[:], in_=st_sb[:])
                tc.strict_bb_all_engine_barrier()
                nc.gpsimd.collective_compute(
                    "AllReduce", mybir.AluOpType.add,
                    replica_groups=[list(range(m.n_cores))],
                    ins=[st_in[:]], outs=[st_out[:]])
                tc.strict_bb_all_engine_barrier()
                st_all = sbS.tile([P, 2], F32, tag="stall")
                nc.sync.dma_start(out=st_all[:], in_=st_out[:])
                nscale = m.N

            mu_c = sbS.tile([P, 1], F32, tag="mu")
            nc.scalar.mul(mu_c[:], st_all[:, 0:1], 1.0 / nscale)
            ex2 = sbS.tile([P, 1], F32, tag="ex2")
            nc.scalar.mul(ex2[:], st_all[:, 1:2], 1.0 / nscale)
            mu2 = sbS.tile([P, 1], F32, tag="mu2")
            nc.scalar.square(mu2[:], mu_c[:])
            var_c = sbS.tile([P, 1], F32, tag="var")
            nc.vector.tensor_sub(var_c[:], ex2[:], mu2[:])
            sd = sbS.tile([P, 1], F32, tag="sd")
            nc.scalar.activation(sd[:], var_c[:],
                                 mybir.ActivationFunctionType.Sqrt,
                                 bias=eps_col[:])
            rsd = sbS.tile([P, 1], F32, tag="rsd")
            nc.vector.reciprocal(rsd[:], sd[:])
            A_c = sbS.tile([P, 1], F32, tag="Ac")
            nc.vector.tensor_mul(A_c[:], gam_col, rsd[:])
            Amu = sbS.tile([P, 1], F32, tag="Amu")
            nc.vector.tensor_mul(Amu[:], A_c[:], mu_c[:])
            B_c = sbS.tile([P, 1], F32, tag="Bc")
            nc.vector.tensor_sub(B_c[:], bet_col, Amu[:])

            A_ps = psT.tile([P, P], F32, tag="pt")
            nc.tensor.transpose(A_ps[:], A_c[:].to_broadcast([P, P]),
                                ident[:])
            A_rep = cst.tile([P, P], F32, tag="Arep")
            nc.scalar.copy(A_rep[:], A_ps[:])
            B_ps = psT.tile([P, P], F32, tag="pt")
            nc.tensor.transpose(B_ps[:], B_c[:].to_broadcast([P, P]),
                                ident[:])
            B_rep = cst.tile([P, P], F32, tag="Brep")
            nc.scalar.copy(B_rep[:], B_ps[:])

            # ================= phase 4: BN apply + relu + residual ========
            for w in range(W):
                rows = P if w < W - 1 else m.LASTR
                t1 = sbS.tile([P, P], F32, tag="t1")
                nc.vector.tensor_mul(t1[:], outpre[w][:], A_rep[:])
                t2 = sbS.tile([P, P], F32, tag="t2")
                nc.vector.tensor_add(t2[:], t1[:], B_rep[:])
                r = sbS.tile([P, P], F32, tag="r")
                nc.scalar.activation(r[:], t2[:],
                                     mybir.ActivationFunctionType.Relu)
                hres = sbS.tile([P, P], F32, tag="hres")
                nc.sync.dma_start(out=hres[:],
                                  in_=hloc[w * P:(w + 1) * P, :])
                o = sbS.tile([P, P], F32, tag="o")
                nc.vector.tensor_add(o[:], r[:], hres[:])
                nc.sync.dma_start(out=out[w * P:w * P + rows, :],
                                  in_=o[:rows, :])
    return nc


def kernel(h, edge_index, W_l, W_r, bias_l, bias_r, att,
           bias_out, gamma, beta):
    n_cores = 8
    meta, in_maps = host_prepare(h, edge_index, W_l, W_r, bias_l, bias_r,
                                 att, bias_out, gamma, beta,
                                 n_cores=n_cores)
    nc = bacc.Bacc()
    build_kernel(nc, meta)
    nc.compile()
    res = run_bass_kernel_spmd(nc, in_maps, core_ids=list(range(n_cores)))
    outs = [res.results[k]["out"] for k in range(n_cores)]
    return np.concatenate(outs, axis=0).astype(np.float32)
